# revision 13
# baseline (speedup 1.0000x reference)
"""Trainium2 Bass kernel for nn_Com_CNN_RNN_18021682774631.

Contract: kernel(**inputs) takes the FULL inputs from reference.setup_inputs()
and returns the FULL [1, 1] float32 output.

Strategy (see spec sharding_hint: batch=1 structurally, weights replicated):
the model is a sequential double-GRU over 256 tokens — there is no batch to
shard, and per-step cross-core collectives dwarf a ~3us step, so every core
runs the identical single-core program on identical inputs (both sentences
batched into the matmul moving dimension) and core 0's output is returned.

v2 redesign (all algebra validated bit-exactly on host, see git history):
  - state is p = h + 1 in bf16, so the t=0 step needs no special case
    (p0 = 1 and Whh@h = Whh@p - rowsum(Whh), with rowsum folded into biases
    host-side).  tanh(x) = 2*sigmoid(2x) - 1, with the doubling pre-folded
    into the n-gate weight rows and biases, so both activations are sigmoids.
  - per step the PE accumulates everything into two psum tiles:
      psRZ[128,8,2] = gi_rz(t)        (one fp32 identity matmul)
                    + Whh_rz @ p      (32 bf16 pairs)
      psN [128,4,2] = c_n2            (one const matmul via indicator rhs)
                    + 2*Whh_n @ p     (16 bf16 pairs)
    and the cell is: rz = sigmoid(psRZ); npre2 = rz_r*psN + 2gi_n(t);
    s = sigmoid(npre2); p' = s*(2-2z) + z*p  — 2 scalar ACTs + 6 short
    vector ops, with zp/omz2 computable right after the first sigmoid so the
    post-matvec critical chain is only ~5 instructions.
  - rz chunks are accumulated before n chunks so the first sigmoid overlaps
    the n-gate matmuls; the two layer scans interleave on the PE so each
    layer's cell tail hides under the other layer's matvec.
  - gru2's input-gate path degenerates (maxpool covers the full range ->
    global max; rows are m*ones) so gi2 = m*rowsum(Wih2) + biases with the
    rowsums precomputed on host.
"""
import os
from contextlib import ExitStack

import numpy as np
import ml_dtypes

import concourse.bass as bass
import concourse.mybir as mybir
import concourse.tile as tile
from concourse.bass_utils import run_bass_kernel_spmd
from concourse.masks import make_identity

dt = mybir.dt
ACT = mybir.ActivationFunctionType
ALU = mybir.AluOpType

# ---------------------------------------------------------------------------
# model dims
E = 512          # embedding/hidden dim of gru1
H = 512          # hidden dim of gru2
G = 3 * E        # 1536 gate width
MC = G // 128    # 12 gate chunks
KC = E // 128    # 4 hidden chunks
NL = 2
T_FULL = 256
TEMP = 256
VOCAB = 50000
N_CORES = 8

W_DT = dt.bfloat16
A_DT = dt.bfloat16
NP_LP = ml_dtypes.bfloat16


# ---------------------------------------------------------------------------
# Workaround for this container's walrus build: InstDrain accepts only ONE
# sync-wait command, but TileContext's exit attaches one wait per active proc
# lane to the final drain.  Split the waits across single-wait NOPs on the
# same sequencer right before the drain (program order preserves semantics).
_PATCHED = False


def _apply_tile_patch():
    global _PATCHED
    if _PATCHED:
        return
    _PATCHED = True
    from concourse.vector_clock import ScopedClock

    def _drain_and_barrier(self, tick_clock, wait_clock):
        nc = self.nc
        probe = nc.sync.nop()
        wait_clock.add_sem_waits(probe.ins, ScopedClock({None: tick_clock.global_clock}))
        waits = list(probe.ins.sync_info.on_wait) if probe.ins.sync_info else []
        if len(waits) > 1:
            probe.ins.sync_info = mybir.SyncInfo(on_wait=[waits[0]], on_update=[])
            for w in waits[1:]:
                extra = nc.sync.nop()
                extra.ins.sync_info = mybir.SyncInfo(on_wait=[w], on_update=[])
        nc.sync.drain()
        nc.all_engine_barrier()
        assert self.sems is not None
        popped = nc._tile_sem_poison_stack.pop()
        assert popped is self._sem_poison
        nc.clear_and_free_semaphores(list(self.sems.allocated().values()))
        nc.all_engine_barrier()

    tile.TileContext._drain_and_barrier = _drain_and_barrier


def _legalize_waits(nc, max_waits=1):
    """This walrus build accepts at most one sync-wait per instruction for
    several opcode structs.  Hoist extra waits onto same-engine NOPs inserted
    immediately before the instruction (same-engine program order makes this
    semantically identical — sem values are monotonic)."""
    import bass_rust

    for f in nc.m.functions:
        for bb in f.blocks:
            idx = 0
            insts = bb.instructions
            while idx < len(insts):
                inst = insts[idx]
                si = getattr(inst, "sync_info", None)
                if si is not None and si.on_wait and len(si.on_wait) > max_waits:
                    waits = list(si.on_wait)
                    keep = waits[:max_waits]
                    extra = waits[max_waits:]
                    inst.sync_info = mybir.SyncInfo(on_wait=keep, on_update=list(si.on_update))
                    for w in extra:
                        nop = bass_rust.InstNoOp(
                            name=nc.get_next_instruction_name(), ins=[], outs=[]
                        )
                        nop.engine = inst.engine
                        nop.sync_info = mybir.SyncInfo(on_wait=[w], on_update=[])
                        nc.register_instruction(nop)
                        insts.insert(idx, nop)
                        idx += 1
                idx += 1


# ---------------------------------------------------------------------------
# host-side weight packing


def _double_n(M):
    """[3E_out, K] gate matrix -> copy with n-rows doubled."""
    M = np.asarray(M, np.float32).copy()
    M[2 * (M.shape[0] // 3):] *= 2.0
    return M


def _pack_lhsT(M):
    """[Gout, K] weight -> [128, K/128, Gout/128, 128] tile array such that
    sb[p, kc, mc, f] = M[mc*128+f, kc*128+p]  (i.e. tiles of M.T)."""
    Mt = np.asarray(M, np.float32).T  # [K, Gout]
    K, Gd = Mt.shape
    return np.ascontiguousarray(
        Mt.reshape(K // 128, 128, Gd // 128, 128).transpose(1, 0, 2, 3)
    ).astype(NP_LP)


def _pack_vec(v):
    """[G] -> [128, G/128]: out[p, mc] = v[mc*128+p]."""
    v = np.asarray(v, np.float32)
    return np.ascontiguousarray(v.reshape(-1, 128).T)


def host_prep(inputs, t_steps=T_FULL):
    """Build the per-core in_map from the full (unsharded) inputs."""
    ip = {k: np.asarray(v, np.float32) if np.asarray(v).dtype != np.int64
          else np.asarray(v) for k, v in inputs.items()}
    m = {}
    m["emb"] = np.ascontiguousarray(ip["emb"].astype(np.float32))
    m["idx"] = np.stack(
        [
            ip["sentA"][:t_steps].astype(np.int32).reshape(-1, 1),
            ip["sentB"][:t_steps].astype(np.int32).reshape(-1, 1),
        ]
    )  # [2, t, 1]

    twoE = 2 * E
    rs_wih = []
    for l in range(NL):
        Whh = np.asarray(ip["Whh1"][l], np.float32)
        Wih = np.asarray(ip["Wih1"][l], np.float32)
        bih = np.asarray(ip["bih1"][l], np.float32)
        bhh = np.asarray(ip["bhh1"][l], np.float32)
        rs_hh = Whh.sum(axis=1)
        rs_ih = Wih.sum(axis=1)
        rs_wih.append(rs_ih)
        m[f"whh1_{l}"] = _pack_lhsT(_double_n(Whh))
        m[f"wih1_{l}"] = _pack_lhsT(_double_n(Wih))
        # gi bias consts: layer 0 epoch 0 consumes raw token embeddings; layer 1
        # (and both layers in epoch 1) consume p-form inputs (x = p - 1).
        b = np.empty(G, np.float32)
        b[:twoE] = bih[:twoE] + bhh[:twoE] - rs_hh[:twoE]
        b[twoE:] = 2.0 * bih[twoE:]
        if l == 1:
            b[:twoE] -= rs_ih[:twoE]
            b[twoE:] -= 2.0 * rs_ih[twoE:]
        m[f"b1f_{l}"] = _pack_vec(b)
        # n-path hh const (goes into psN via the indicator matmul)
        m[f"cn_{l}"] = np.ascontiguousarray(
            (2.0 * (bhh[twoE:] - rs_hh[twoE:])).reshape(KC, 128)
        )
    # epoch-1 layer-0 gi consts (p-form input)
    Whh0 = np.asarray(ip["Whh1"][0], np.float32)
    bih0 = np.asarray(ip["bih1"][0], np.float32)
    bhh0 = np.asarray(ip["bhh1"][0], np.float32)
    rs_hh0 = Whh0.sum(axis=1)
    e0 = np.empty(G, np.float32)
    e0[:twoE] = bih0[:twoE] + bhh0[:twoE] - rs_hh0[:twoE] - rs_wih[0][:twoE]
    e0[twoE:] = 2.0 * (bih0[twoE:] - rs_wih[0][twoE:])
    m["e0f"] = _pack_vec(e0)

    # indicator rhs for the psN const matmul: ind[c, c'*2+s] = (c == c')
    ind = np.zeros((KC, KC * 2), np.float32)
    for c in range(KC):
        ind[c, 2 * c] = 1.0
        ind[c, 2 * c + 1] = 1.0
    m["ind"] = ind

    # gru2: gi2 = m * rowsum(Wih2) + biases (host-precomputed rowsums)
    Wih2 = np.asarray(ip["Wih2"], np.float32)
    Whh2 = np.asarray(ip["Whh2"], np.float32)
    bih2 = np.asarray(ip["bih2"], np.float32)
    bhh2 = np.asarray(ip["bhh2"], np.float32)
    rs_ih2 = Wih2.sum(axis=1)
    rs_hh2 = Whh2.sum(axis=1)
    m["whh2"] = _pack_lhsT(_double_n(Whh2))
    s2 = np.empty(G, np.float32)
    s2[:twoE] = rs_ih2[:twoE]
    s2[twoE:] = 2.0 * rs_ih2[twoE:]
    m["s2f"] = _pack_vec(s2)
    c2 = np.empty(G, np.float32)
    c2[:twoE] = bih2[:twoE] + bhh2[:twoE] - rs_hh2[:twoE]
    c2[twoE:] = 2.0 * bih2[twoE:]
    m["c2f"] = _pack_vec(c2)
    m["cn_2"] = np.ascontiguousarray(
        (2.0 * (bhh2[twoE:] - rs_hh2[twoE:])).reshape(KC, 128)
    )

    # conv: wc[p, i*4+kc, o] = conv_w[o, i, kc*128+p]
    cw = np.asarray(ip["conv_w"], np.float32)  # [2, 2, 512]
    wc = cw.reshape(2, 2, 4, 128).transpose(3, 1, 2, 0).reshape(128, 8, 2)
    m["wc"] = np.ascontiguousarray(wc).astype(NP_LP)
    m["convb"] = np.asarray(ip["conv_b"], np.float32).reshape(2, 1)
    # double linear: hs = hx @ WA + hv @ WB + b_bi ; WA is [H, TEMP] = [K, M]
    m["wa"] = _pack_lhsT(ip["WA"].T)
    m["wb"] = _pack_lhsT(ip["WB"].T)
    m["bbi"] = _pack_vec(ip["b_bi"])  # [128, 2]
    # W_lin [1, 256]: wlin[p, kc, 0] = W_lin[0, kc*128+p]
    m["wlin"] = np.ascontiguousarray(
        np.asarray(ip["W_lin"], np.float32).reshape(2, 128).T.reshape(128, 2, 1)
    ).astype(NP_LP)
    m["blin"] = np.asarray(ip["b_lin"], np.float32).reshape(1, 1)
    return m


# ---------------------------------------------------------------------------
# device program


def _bcast(ap, extra):
    """append broadcast dims (step 0) to an AP"""
    return bass.AP(tensor=ap.tensor, offset=ap.offset, ap=list(ap.ap) + [[0, n] for n in extra])


def build_nc(t_steps=T_FULL, batch=16):
    _apply_tile_patch()
    assert t_steps % batch == 0
    lag = batch + 1
    nc = bass.Bass()

    def dparam(name, shape, dtype):
        return nc.declare_dram_parameter(name, list(shape), dtype, isOutput=False)

    emb = dparam("emb", [VOCAB, E], dt.float32)
    idx = dparam("idx", [2, t_steps, 1], dt.int32)
    w1d = [
        (dparam(f"wih1_{l}", [128, KC, MC, 128], W_DT), dparam(f"whh1_{l}", [128, KC, MC, 128], W_DT))
        for l in range(NL)
    ]
    b1d = [dparam(f"b1f_{l}", [128, MC], dt.float32) for l in range(NL)]
    cnd = [dparam(f"cn_{l}", [KC, 128], dt.float32) for l in range(NL)]
    e0f_d = dparam("e0f", [128, MC], dt.float32)
    ind_d = dparam("ind", [KC, KC * 2], dt.float32)
    whh2_d = dparam("whh2", [128, KC, MC, 128], W_DT)
    s2f_d = dparam("s2f", [128, MC], dt.float32)
    c2f_d = dparam("c2f", [128, MC], dt.float32)
    cn2_d = dparam("cn_2", [KC, 128], dt.float32)
    wc_d = dparam("wc", [128, 8, 2], W_DT)
    convb_d = dparam("convb", [2, 1], dt.float32)
    wa_d = dparam("wa", [128, KC, 2, 128], W_DT)
    wb_d = dparam("wb", [128, KC, 2, 128], W_DT)
    bbi_d = dparam("bbi", [128, 2], dt.float32)
    wlin_d = dparam("wlin", [128, 2, 1], W_DT)
    blin_d = dparam("blin", [1, 1], dt.float32)
    out_d = nc.declare_dram_parameter("out", [1, 1], dt.float32, isOutput=True)

    th_cnt = max(1, t_steps // 128)
    tok_pp = min(128, t_steps)  # tokens per indirect gather

    with tile.TileContext(nc) as tc, ExitStack() as ctx:
        P = ctx.enter_context(tc.tile_pool(name="persist", bufs=1))
        Wp = ctx.enter_context(tc.tile_pool(name="work", bufs=3))
        HP = ctx.enter_context(tc.tile_pool(name="hstate", bufs=3))
        DP = ctx.enter_context(tc.tile_pool(name="dram", bufs=1, space="DRAM"))

        # ---- input-dependent DMAs first so the gather isn't queued behind
        # ~8MB of weights ----
        idx_sb = P.tile([tok_pp, 2, th_cnt], dt.int32, tag="idx")
        for s in range(2):
            nc.gpsimd.dma_start(
                out=idx_sb[:, s, :],
                in_=idx[s, :, :].rearrange("(h p) o -> p (h o)", p=tok_pp),
            )
        gat = P.tile([tok_pp, th_cnt, 2, E], dt.float32, tag="gat")
        for s in range(2):
            for h in range(th_cnt):
                nc.gpsimd.indirect_dma_start(
                    out=gat[:, h, s, :],
                    out_offset=None,
                    in_=emb[:],
                    in_offset=bass.IndirectOffsetOnAxis(ap=idx_sb[:, s, h : h + 1], axis=0),
                )

        # ---- persistent SBUF: layer-0 weights first (needed first) ----
        w1_sb = []
        b1_sb = []
        cn_sb = []
        for l in range(NL):
            wi = P.tile([128, KC, MC, 128], W_DT, tag=f"wih1_{l}")
            wh = P.tile([128, KC, MC, 128], W_DT, tag=f"whh1_{l}")
            nc.gpsimd.dma_start(out=wi[:], in_=w1d[l][0][:])
            nc.gpsimd.dma_start(out=wh[:], in_=w1d[l][1][:])
            w1_sb.append((wi, wh))
            bf = P.tile([128, MC], dt.float32, tag=f"b1f_{l}")
            nc.gpsimd.dma_start(out=bf[:], in_=b1d[l][:])
            b1_sb.append(bf)
            cn = P.tile([KC, 128], dt.float32, tag=f"cn_{l}")
            nc.gpsimd.dma_start(out=cn[:], in_=cnd[l][:])
            cn_sb.append(cn)
        ind_sb = P.tile([KC, KC * 2], dt.float32, tag="ind")
        nc.gpsimd.dma_start(out=ind_sb[:], in_=ind_d[:])
        e0f_sb = P.tile([128, MC], dt.float32, tag="e0f")
        nc.gpsimd.dma_start(out=e0f_sb[:], in_=e0f_d[:])
        whh2_sb = P.tile([128, KC, MC, 128], W_DT, tag="whh2")
        nc.gpsimd.dma_start(out=whh2_sb[:], in_=whh2_d[:])
        s2f_sb = P.tile([128, MC], dt.float32, tag="s2f")
        nc.gpsimd.dma_start(out=s2f_sb[:], in_=s2f_d[:])
        c2f_sb = P.tile([128, MC], dt.float32, tag="c2f")
        nc.gpsimd.dma_start(out=c2f_sb[:], in_=c2f_d[:])
        cn2_sb = P.tile([KC, 128], dt.float32, tag="cn_2")
        nc.gpsimd.dma_start(out=cn2_sb[:], in_=cn2_d[:])
        wc_sb = P.tile([128, 8, 2], W_DT, tag="wc")
        nc.gpsimd.dma_start(out=wc_sb[:], in_=wc_d[:])
        convb_sb = P.tile([2, 1], dt.float32, tag="convb")
        nc.gpsimd.dma_start(out=convb_sb[:], in_=convb_d[:])
        wa_sb = P.tile([128, KC, 2, 128], W_DT, tag="wa")
        wb_sb = P.tile([128, KC, 2, 128], W_DT, tag="wb")
        nc.gpsimd.dma_start(out=wa_sb[:], in_=wa_d[:])
        nc.gpsimd.dma_start(out=wb_sb[:], in_=wb_d[:])
        bbi_sb = P.tile([128, 2], dt.float32, tag="bbi")
        nc.gpsimd.dma_start(out=bbi_sb[:], in_=bbi_d[:])
        wlin_sb = P.tile([128, 2, 1], W_DT, tag="wlin")
        nc.gpsimd.dma_start(out=wlin_sb[:], in_=wlin_d[:])
        blin_sb = P.tile([1, 1], dt.float32, tag="blin")
        nc.gpsimd.dma_start(out=blin_sb[:], in_=blin_d[:])

        ident = P.tile([128, 128], dt.float32, tag="ident")
        make_identity(nc, ident[:])
        ones_col = P.tile([128, 1], A_DT, tag="ones_col")
        nc.vector.memset(ones_col[:], 1.0)
        ones2 = P.tile([2, 128], W_DT, tag="ones2")
        nc.vector.memset(ones2[:], 1.0)
        pinit = P.tile([128, KC, 2], A_DT, tag="pinit")
        nc.vector.memset(pinit[:], 1.0)

        xT = P.tile([128, KC, 2, t_steps], A_DT, tag="xT")
        gi0 = P.tile([128, MC, 2, t_steps], dt.float32, tag="gi0")
        p0h = P.tile([128, KC, 2, t_steps], A_DT, tag="p0h")
        gi1 = P.tile([128, 2, MC, 2, batch], dt.float32, tag="gi1")

        # ================= phase A: transpose + gi0 =================
        with tc.tile_pool(name="psA", bufs=2, space="PSUM") as psA:
            for s in range(2):
                for h in range(th_cnt):
                    for c in range(KC):
                        tp = psA.tile([128, tok_pp], dt.float32, tag="tr")
                        nc.tensor.transpose(
                            out=tp[:],
                            in_=gat[:, h, s, c * 128 : (c + 1) * 128],
                            identity=ident[:tok_pp, :tok_pp],
                        )
                        nc.vector.tensor_copy(
                            out=xT[:, c, s, h * 128 : h * 128 + tok_pp], in_=tp[:]
                        )
            # gi0 = Wih1[0](n-doubled) @ x + b1f_0, gate-major
            for mc in range(MC):
                gp = psA.tile([128, 2, t_steps], dt.float32, tag="gi0p")
                for kc in range(KC):
                    nc.tensor.matmul(
                        out=gp[:],
                        lhsT=w1_sb[0][0][:, kc, mc, :],
                        rhs=xT[:, kc, :, :],
                        start=(kc == 0),
                        stop=(kc == KC - 1),
                    )
                nc.vector.tensor_scalar(
                    out=gi0[:, mc, :, :],
                    in0=gp[:],
                    scalar1=b1_sb[0][:, mc : mc + 1],
                    scalar2=None,
                    op0=ALU.add,
                )

        # ================= the fused GRU step =================
        def step_emit(PS, whh_sb, cn_t, gi_r_ap, gi_z_ap, gi_n2_ap, p_prev_fn,
                      p_prev_full, p_out_ap, tag, p_out_extra=None):
            """One GRU step for both sentences.

            PE order R -> N -> Z with three separate psum tiles so sigma_r can
            fire a third of the way into the matvec and the n-path chain
            (rhn2 -> npre2 -> sigma2 -> u -> p') starts as soon as psN lands.
            Scalar queue order is sigma_r, sigma_z, sigma2; vector ops are
            emitted in expected-ready order (the engines run in-order)."""
            psR = PS.tile([128, 4, 2], dt.float32, tag="r")
            psZ = PS.tile([128, 4, 2], dt.float32, tag="z")
            psN = PS.tile([128, 4, 2], dt.float32, tag="n")
            nc.tensor.matmul(out=psR[:], lhsT=ident[:], rhs=gi_r_ap,
                             start=True, stop=True, skip_group_check=True)
            for mc in range(4):
                for kc in range(KC):
                    nc.tensor.matmul(
                        out=psR[:, mc, :], lhsT=whh_sb[:, kc, mc, :],
                        rhs=p_prev_fn(kc), start=False, stop=(kc == KC - 1),
                        skip_group_check=True,
                    )
            r_sb = Wp.tile([128, 4, 2], dt.float32, tag=f"sr{tag}")
            nc.scalar.activation(r_sb[:], psR[:], ACT.Sigmoid)
            nc.tensor.matmul(out=psN[:], lhsT=cn_t[:], rhs=ind_sb[:],
                             start=True, stop=True, skip_group_check=True)
            for mc in range(4):
                for kc in range(KC):
                    nc.tensor.matmul(
                        out=psN[:, mc, :], lhsT=whh_sb[:, kc, 8 + mc, :],
                        rhs=p_prev_fn(kc), start=False, stop=(kc == KC - 1),
                        skip_group_check=True,
                    )
            nc.tensor.matmul(out=psZ[:], lhsT=ident[:], rhs=gi_z_ap,
                             start=True, stop=True, skip_group_check=True)
            for mc in range(4):
                for kc in range(KC):
                    nc.tensor.matmul(
                        out=psZ[:, mc, :], lhsT=whh_sb[:, kc, 4 + mc, :],
                        rhs=p_prev_fn(kc), start=False, stop=(kc == KC - 1),
                        skip_group_check=True,
                    )
            z_sb = Wp.tile([128, 4, 2], dt.float32, tag=f"sz{tag}")
            nc.scalar.activation(z_sb[:], psZ[:], ACT.Sigmoid)
            rhn2 = Wp.tile([128, 4, 2], dt.float32, tag=f"rh{tag}")
            nc.vector.tensor_tensor(out=rhn2[:], in0=psN[:], in1=r_sb[:], op=ALU.mult)
            npre2 = Wp.tile([128, 4, 2], dt.float32, tag=f"np{tag}")
            nc.vector.tensor_tensor(out=npre2[:], in0=rhn2[:], in1=gi_n2_ap, op=ALU.add)
            zp = Wp.tile([128, 4, 2], dt.float32, tag=f"zp{tag}")
            nc.vector.tensor_tensor(out=zp[:], in0=z_sb[:], in1=p_prev_full, op=ALU.mult)
            omz2 = Wp.tile([128, 4, 2], dt.float32, tag=f"om{tag}")
            nc.vector.tensor_scalar(
                out=omz2[:], in0=z_sb[:], scalar1=-2.0, scalar2=2.0,
                op0=ALU.mult, op1=ALU.add,
            )
            sg = Wp.tile([128, 4, 2], dt.float32, tag=f"s{tag}")
            nc.scalar.activation(sg[:], npre2[:], ACT.Sigmoid)
            u = Wp.tile([128, 4, 2], dt.float32, tag=f"u{tag}")
            nc.vector.tensor_tensor(out=u[:], in0=sg[:], in1=omz2[:], op=ALU.mult)
            nc.vector.tensor_tensor(out=p_out_ap, in0=u[:], in1=zp[:], op=ALU.add)
            if p_out_extra is not None:
                nc.vector.tensor_tensor(out=p_out_extra, in0=u[:], in1=zp[:], op=ALU.add)

        # ================= phase B: the two interleaved scans =================
        p1_ring = [None]
        with tc.tile_pool(name="psB", bufs=2, space="PSUM") as psB:

            def l0_step(t):
                prev = pinit if t == 0 else None
                if prev is None:
                    pfn = lambda kc: p0h[:, kc, :, t - 1]
                    pfull = p0h[:, :, :, t - 1]
                else:
                    pfn = lambda kc: prev[:, kc, :]
                    pfull = prev[:, :, :]
                step_emit(
                    psB, w1_sb[0][1], cn_sb[0],
                    gi0[:, 0:4, :, t], gi0[:, 4:8, :, t], gi0[:, 8:12, :, t],
                    pfn, pfull, p0h[:, :, :, t], "a",
                )

            def gi1_batch(b):
                t0 = b * batch
                gp = psB.tile([128, MC, 2, batch], dt.float32, tag="g1")
                for mc in range(MC):
                    for kc in range(KC):
                        nc.tensor.matmul(
                            out=gp[:, mc, :, :],
                            lhsT=w1_sb[1][0][:, kc, mc, :],
                            rhs=p0h[:, kc, :, t0 : t0 + batch],
                            start=(kc == 0),
                            stop=(kc == KC - 1),
                        )
                nc.vector.tensor_tensor(
                    out=gi1[:, b % 2, :, :, :],
                    in0=gp[:],
                    in1=_bcast(b1_sb[1][:, :], [2, batch]),
                    op=ALU.add,
                )

            def l1_step(t):
                buf = (t // batch) % 2
                tb = t % batch
                prev = pinit if t == 0 else p1_ring[0]
                pn = HP.tile([128, KC, 2], A_DT, tag="p1")
                step_emit(
                    psB, w1_sb[1][1], cn_sb[1],
                    gi1[:, buf, 0:4, :, tb], gi1[:, buf, 4:8, :, tb],
                    gi1[:, buf, 8:12, :, tb],
                    lambda kc: prev[:, kc, :], prev[:, :, :], pn[:], "b",
                )
                p1_ring[0] = pn

            # Per-step sim-time floors (tile_wait_until) pin the scheduler to
            # the intended slot cadence: the Tile list-scheduler's cost model
            # underestimates the PE stream (ldweights is free there), so
            # without floors it queues step t+1's rz-sigmoid ahead of step
            # t's n-sigmoid on the in-order scalar engine, head-of-line
            # blocking every cell chain on real hardware.
            SLOT_MS = 0.003
            for t in range(t_steps):
                with tc.tile_wait_until(t * SLOT_MS):
                    l0_step(t)
                    if t % batch == batch - 1:
                        gi1_batch(t // batch)
                with tc.tile_wait_until((t + 0.5) * SLOT_MS):
                    if t >= lag:
                        l1_step(t - lag)
            for i, tp in enumerate(range(t_steps - lag, t_steps)):
                with tc.tile_wait_until((t_steps + i) * SLOT_MS):
                    l1_step(tp)

            # ============ epoch 1: seq len 2 over the finals (p-form) ============
            e1p = P.tile([128, KC, 2, 2], A_DT, tag="e1p")  # [.., step, sent]
            nc.vector.tensor_copy(out=e1p[:, :, 0, :], in_=p0h[:, :, :, t_steps - 1])
            nc.vector.tensor_copy(out=e1p[:, :, 1, :], in_=p1_ring[0][:, :, :])
            pf = []
            xcur = e1p
            for l in range(NL):
                gp = psB.tile([128, MC, 2, 2], dt.float32, tag="g1")
                for mc in range(MC):
                    for kc in range(KC):
                        nc.tensor.matmul(
                            out=gp[:, mc, :, :],
                            lhsT=w1_sb[l][0][:, kc, mc, :],
                            rhs=xcur[:, kc, :, :],
                            start=(kc == 0),
                            stop=(kc == KC - 1),
                        )
                gie = P.tile([128, MC, 2, 2], dt.float32, tag=f"gie{l}")
                bsrc = e0f_sb if l == 0 else b1_sb[1]
                nc.vector.tensor_tensor(
                    out=gie[:], in0=gp[:], in1=_bcast(bsrc[:, :], [2, 2]), op=ALU.add
                )
                xnext = P.tile([128, KC, 2, 2], A_DT, tag=f"e1y{l}")
                pe1f = HP.tile([128, KC, 2], A_DT, tag=f"pe{l}")
                step_emit(
                    psB, w1_sb[l][1], cn_sb[l],
                    gie[:, 0:4, 0, :], gie[:, 4:8, 0, :], gie[:, 8:12, 0, :],
                    lambda kc: pinit[:, kc, :], pinit[:, :, :],
                    xnext[:, :, 0, :], f"e{l}",
                )
                step_emit(
                    psB, w1_sb[l][1], cn_sb[l],
                    gie[:, 0:4, 1, :], gie[:, 4:8, 1, :], gie[:, 8:12, 1, :],
                    lambda kc: xnext[:, kc, 0, :], xnext[:, :, 0, :],
                    xnext[:, :, 1, :], f"e{l}", p_out_extra=pe1f[:],
                )
                pf.append(pe1f)
                xcur = xnext

            # hE[i] = pE[i] - 1  (fp32)
            hE = []
            for i in range(2):
                he = Wp.tile([128, KC, 2], dt.float32, tag=f"hE{i}")
                nc.vector.tensor_scalar(
                    out=he[:], in0=pf[i][:], scalar1=-1.0, scalar2=None, op0=ALU.add
                )
                hE.append(he)

        # ================= phase C: conv + maxpool + gru2 + head =================
        with tc.tile_pool(name="psC1", bufs=1, space="PSUM") as psC:
            PADL = 255
            hp_dram = DP.tile([4, E + 2 * PADL], A_DT)  # rows (i, s)
            zs = P.tile([4, E + 2 * PADL], A_DT, tag="zs")
            nc.vector.memset(zs[:], 0.0)
            nc.gpsimd.dma_start(out=hp_dram[:], in_=zs[:])
            # flatten hE via PE transpose, write rows straight into the pad buffer
            for i in range(2):
                for s in range(2):
                    tp = psC.tile([KC, 128], dt.float32, tag="tr2")
                    nc.tensor.transpose(out=tp[:], in_=hE[i][:, :, s], identity=ident[:])
                    trs = Wp.tile([KC, 128], A_DT, tag="trs")
                    nc.vector.tensor_copy(out=trs[:], in_=tp[:])
                    r = 2 * i + s
                    nc.gpsimd.dma_start(
                        out=hp_dram[r : r + 1, PADL : PADL + E].rearrange(
                            "o (c f) -> (o c) f", c=KC
                        ),
                        in_=trs[:],
                    )
            # im2col: load time-major rows Hrow[t, k] = hp[2t + k] (contiguous
            # along k), then PE-transpose to K-major Hcol[(i,k), (s,t)].
            Hcol = P.tile([128, 8, 2, 256], A_DT, tag="Hcol")
            ident_lp = P.tile([128, 128], A_DT, tag="ident_lp")
            make_identity(nc, ident_lp[:])
            hp_base = hp_dram[:]
            row_sz = E + 2 * PADL
            for i in range(2):
                for s in range(2):
                    for th in range(2):
                        hrow = Wp.tile([128, E], A_DT, tag="Hrow")
                        src = bass.AP(
                            tensor=hp_base.tensor,
                            offset=hp_base.offset + (i * 2 + s) * row_sz + 256 * th,
                            ap=[[2, 128], [1, E]],
                        )
                        nc.gpsimd.dma_start(out=hrow[:], in_=src)
                        for kc in range(KC):
                            tpb = psC.tile([128, 128], A_DT, tag="tr3")
                            nc.tensor.transpose(
                                out=tpb[:],
                                in_=hrow[:, kc * 128 : (kc + 1) * 128],
                                identity=ident_lp[:],
                            )
                            nc.vector.tensor_copy(
                                out=Hcol[:, i * KC + kc, s, th * 128 : (th + 1) * 128],
                                in_=tpb[:],
                            )
            # conv matmul -> [2(out_ch), 2(s), 256(t)] then global max per (o, s)
            cp = psC.tile([2, 2, 256], dt.float32, tag="conv")
            for ck in range(8):
                nc.tensor.matmul(
                    out=cp[:],
                    lhsT=wc_sb[:, ck, :],
                    rhs=Hcol[:, ck, :, :],
                    start=(ck == 0),
                    stop=(ck == 7),
                )
            mx = Wp.tile([2, 2, 1], dt.float32, tag="mx")
            nc.vector.tensor_reduce(out=mx[:], in_=cp[:], axis=mybir.AxisListType.X, op=ALU.max)
            m_sb = Wp.tile([2, 2], dt.float32, tag="m_sb")
            nc.vector.tensor_scalar(
                out=m_sb[:],
                in0=mx[:, :, 0],
                scalar1=convb_sb[:, 0:1],
                scalar2=None,
                op0=ALU.add,
            )
            # broadcast m over partitions via ones-matmul with a diagonal rhs
            m_lp = Wp.tile([2, 2], A_DT, tag="m_lp")
            nc.vector.tensor_copy(out=m_lp[:], in_=m_sb[:])
            md = Wp.tile([2, 4], A_DT, tag="md")
            nc.vector.memset(md[:], 0.0)
            # diagonal placement via DMA (compute engines can't address odd
            # base partitions)
            nc.gpsimd.dma_start(out=md[0:1, 0:2], in_=m_lp[0:1, :])
            nc.gpsimd.dma_start(out=md[1:2, 2:4], in_=m_lp[1:2, :])
            mp = psC.tile([128, 4], dt.float32, tag="mbc")
            nc.tensor.matmul(out=mp[:], lhsT=ones2[:], rhs=md[:], start=True, stop=True)
            mB = Wp.tile([128, 4], dt.float32, tag="mB")
            nc.vector.tensor_copy(out=mB[:], in_=mp[:])

        with tc.tile_pool(name="psC2", bufs=1, space="PSUM") as psC:
            # gi2[tp] = m[tp] * s2f + c2f  (both gates pre-scaled; n-part doubled)
            gi2 = P.tile([128, MC, 2, 2], dt.float32, tag="gi2")  # [.., step, sent]
            for tpp in range(2):
                for s in range(2):
                    nc.vector.scalar_tensor_tensor(
                        out=gi2[:, :, tpp, s],
                        in0=s2f_sb[:],
                        scalar=mB[:, 2 * tpp + s : 2 * tpp + s + 1],
                        in1=c2f_sb[:],
                        op0=ALU.mult,
                        op1=ALU.add,
                    )
            # gru2: 2 steps
            p2a = HP.tile([128, KC, 2], A_DT, tag="p2a")
            step_emit(
                psC, whh2_sb, cn2_sb,
                gi2[:, 0:4, 0, :], gi2[:, 4:8, 0, :], gi2[:, 8:12, 0, :],
                lambda kc: pinit[:, kc, :], pinit[:, :, :], p2a[:], "g2",
            )
            p2b = HP.tile([128, KC, 2], A_DT, tag="p2b")
            step_emit(
                psC, whh2_sb, cn2_sb,
                gi2[:, 0:4, 1, :], gi2[:, 4:8, 1, :], gi2[:, 8:12, 1, :],
                lambda kc: p2a[:, kc, :], p2a[:, :, :], p2b[:], "g2",
            )
            h2 = Wp.tile([128, KC, 2], dt.float32, tag="h2")
            nc.vector.tensor_scalar(
                out=h2[:], in0=p2b[:], scalar1=-1.0, scalar2=None, op0=ALU.add
            )
            # head: hx = hA*hB, hv = |hA-hB|
            hx = Wp.tile([128, KC], dt.float32, tag="hx")
            nc.vector.tensor_tensor(out=hx[:], in0=h2[:, :, 0], in1=h2[:, :, 1], op=ALU.mult)
            hv0 = Wp.tile([128, KC], dt.float32, tag="hv0")
            nc.vector.tensor_tensor(out=hv0[:], in0=h2[:, :, 0], in1=h2[:, :, 1], op=ALU.subtract)
            hv = Wp.tile([128, KC], dt.float32, tag="hv")
            nc.scalar.activation(hv[:], hv0[:], ACT.Abs)
            hx_lp = Wp.tile([128, KC], A_DT, tag="hx_lp")
            hv_lp = Wp.tile([128, KC], A_DT, tag="hv_lp")
            nc.vector.tensor_copy(out=hx_lp[:], in_=hx[:])
            nc.vector.tensor_copy(out=hv_lp[:], in_=hv[:])
            hsp = psC.tile([128, 2], dt.float32, tag="hs")
            for mc in range(2):
                for kc in range(KC):
                    nc.tensor.matmul(
                        out=hsp[:, mc : mc + 1],
                        lhsT=wa_sb[:, kc, mc, :],
                        rhs=hx_lp[:, kc : kc + 1],
                        start=(kc == 0),
                        stop=False,
                    )
                for kc in range(KC):
                    nc.tensor.matmul(
                        out=hsp[:, mc : mc + 1],
                        lhsT=wb_sb[:, kc, mc, :],
                        rhs=hv_lp[:, kc : kc + 1],
                        start=False,
                        stop=(kc == KC - 1),
                    )
            hspre = Wp.tile([128, 2], dt.float32, tag="hspre")
            nc.vector.tensor_tensor(out=hspre[:], in0=hsp[:], in1=bbi_sb[:], op=ALU.add)
            ht = Wp.tile([128, 2], dt.float32, tag="ht")
            nc.scalar.activation(ht[:], hspre[:], ACT.Tanh)
            ht_lp = Wp.tile([128, 2], A_DT, tag="ht_lp")
            nc.vector.tensor_copy(out=ht_lp[:], in_=ht[:])
            op = psC.tile([1, 1], dt.float32, tag="out")
            for kc in range(2):
                nc.tensor.matmul(
                    out=op[:],
                    lhsT=wlin_sb[:, kc, :],
                    rhs=ht_lp[:, kc : kc + 1],
                    start=(kc == 0),
                    stop=(kc == 1),
                )
            out_sb = Wp.tile([1, 1], dt.float32, tag="osb")
            nc.scalar.activation(out_sb[:], op[:], ACT.Sigmoid, bias=blin_sb[:])
            nc.gpsimd.dma_start(out=out_d[:], in_=out_sb[:])

    _legalize_waits(nc)
    return nc


# ---------------------------------------------------------------------------
_NC_CACHE = {}


def _get_nc(t_steps=T_FULL, batch=16):
    key = (t_steps, batch)
    if key not in _NC_CACHE:
        _NC_CACHE[key] = build_nc(t_steps, batch)
    return _NC_CACHE[key]


def run(inputs, t_steps=T_FULL, batch=16, trace=False):
    nc = _get_nc(t_steps, batch)
    in_map = host_prep(inputs, t_steps)
    res = run_bass_kernel_spmd(nc, [in_map] * N_CORES, list(range(N_CORES)), trace=trace)
    out = np.asarray(res.results[0]["out"], np.float32)
    return out, res


def kernel(**inputs) -> np.ndarray:
    out, _ = run(inputs)
    return out


# revision 14
# speedup vs baseline: 1.0202x; 1.0202x over previous
"""Trainium2 Bass kernel for nn_Com_CNN_RNN_18021682774631.

Contract: kernel(**inputs) takes the FULL inputs from reference.setup_inputs()
and returns the FULL [1, 1] float32 output.

Strategy (see spec sharding_hint: batch=1 structurally, weights replicated):
the model is a sequential double-GRU over 256 tokens — there is no batch to
shard, and per-step cross-core collectives dwarf a ~3us step, so every core
runs the identical single-core program on identical inputs (both sentences
batched into the matmul moving dimension) and core 0's output is returned.

v2 redesign (all algebra validated bit-exactly on host, see git history):
  - state is p = h + 1 in bf16, so the t=0 step needs no special case
    (p0 = 1 and Whh@h = Whh@p - rowsum(Whh), with rowsum folded into biases
    host-side).  tanh(x) = 2*sigmoid(2x) - 1, with the doubling pre-folded
    into the n-gate weight rows and biases, so both activations are sigmoids.
  - per step the PE accumulates everything into two psum tiles:
      psRZ[128,8,2] = gi_rz(t)        (one fp32 identity matmul)
                    + Whh_rz @ p      (32 bf16 pairs)
      psN [128,4,2] = c_n2            (one const matmul via indicator rhs)
                    + 2*Whh_n @ p     (16 bf16 pairs)
    and the cell is: rz = sigmoid(psRZ); npre2 = rz_r*psN + 2gi_n(t);
    s = sigmoid(npre2); p' = s*(2-2z) + z*p  — 2 scalar ACTs + 6 short
    vector ops, with zp/omz2 computable right after the first sigmoid so the
    post-matvec critical chain is only ~5 instructions.
  - rz chunks are accumulated before n chunks so the first sigmoid overlaps
    the n-gate matmuls; the two layer scans interleave on the PE so each
    layer's cell tail hides under the other layer's matvec.
  - gru2's input-gate path degenerates (maxpool covers the full range ->
    global max; rows are m*ones) so gi2 = m*rowsum(Wih2) + biases with the
    rowsums precomputed on host.
"""
import os
from contextlib import ExitStack

import numpy as np
import ml_dtypes

import concourse.bass as bass
import concourse.mybir as mybir
import concourse.tile as tile
from concourse.bass_utils import run_bass_kernel_spmd
from concourse.masks import make_identity

dt = mybir.dt
ACT = mybir.ActivationFunctionType
ALU = mybir.AluOpType

# ---------------------------------------------------------------------------
# model dims
E = 512          # embedding/hidden dim of gru1
H = 512          # hidden dim of gru2
G = 3 * E        # 1536 gate width
MC = G // 128    # 12 gate chunks
KC = E // 128    # 4 hidden chunks
NL = 2
T_FULL = 256
TEMP = 256
VOCAB = 50000
N_CORES = 8

W_DT = dt.bfloat16
A_DT = dt.bfloat16
NP_LP = ml_dtypes.bfloat16


# ---------------------------------------------------------------------------
# Workaround for this container's walrus build: InstDrain accepts only ONE
# sync-wait command, but TileContext's exit attaches one wait per active proc
# lane to the final drain.  Split the waits across single-wait NOPs on the
# same sequencer right before the drain (program order preserves semantics).
_PATCHED = False


def _apply_tile_patch():
    global _PATCHED
    if _PATCHED:
        return
    _PATCHED = True
    from concourse.vector_clock import ScopedClock

    def _drain_and_barrier(self, tick_clock, wait_clock):
        nc = self.nc
        probe = nc.sync.nop()
        wait_clock.add_sem_waits(probe.ins, ScopedClock({None: tick_clock.global_clock}))
        waits = list(probe.ins.sync_info.on_wait) if probe.ins.sync_info else []
        if len(waits) > 1:
            probe.ins.sync_info = mybir.SyncInfo(on_wait=[waits[0]], on_update=[])
            for w in waits[1:]:
                extra = nc.sync.nop()
                extra.ins.sync_info = mybir.SyncInfo(on_wait=[w], on_update=[])
        nc.sync.drain()
        nc.all_engine_barrier()
        assert self.sems is not None
        popped = nc._tile_sem_poison_stack.pop()
        assert popped is self._sem_poison
        nc.clear_and_free_semaphores(list(self.sems.allocated().values()))
        nc.all_engine_barrier()

    tile.TileContext._drain_and_barrier = _drain_and_barrier


def _legalize_waits(nc, max_waits=1):
    """This walrus build accepts at most one sync-wait per instruction for
    several opcode structs.  Hoist extra waits onto same-engine NOPs inserted
    immediately before the instruction (same-engine program order makes this
    semantically identical — sem values are monotonic)."""
    import bass_rust

    for f in nc.m.functions:
        for bb in f.blocks:
            idx = 0
            insts = bb.instructions
            while idx < len(insts):
                inst = insts[idx]
                si = getattr(inst, "sync_info", None)
                if si is not None and si.on_wait and len(si.on_wait) > max_waits:
                    waits = list(si.on_wait)
                    keep = waits[:max_waits]
                    extra = waits[max_waits:]
                    inst.sync_info = mybir.SyncInfo(on_wait=keep, on_update=list(si.on_update))
                    for w in extra:
                        nop = bass_rust.InstNoOp(
                            name=nc.get_next_instruction_name(), ins=[], outs=[]
                        )
                        nop.engine = inst.engine
                        nop.sync_info = mybir.SyncInfo(on_wait=[w], on_update=[])
                        nc.register_instruction(nop)
                        insts.insert(idx, nop)
                        idx += 1
                idx += 1


# ---------------------------------------------------------------------------
# host-side weight packing


def _double_n(M):
    """[3E_out, K] gate matrix -> copy with n-rows doubled."""
    M = np.asarray(M, np.float32).copy()
    M[2 * (M.shape[0] // 3):] *= 2.0
    return M


def _pack_lhsT(M):
    """[Gout, K] weight -> [128, K/128, Gout/128, 128] tile array such that
    sb[p, kc, mc, f] = M[mc*128+f, kc*128+p]  (i.e. tiles of M.T)."""
    Mt = np.asarray(M, np.float32).T  # [K, Gout]
    K, Gd = Mt.shape
    return np.ascontiguousarray(
        Mt.reshape(K // 128, 128, Gd // 128, 128).transpose(1, 0, 2, 3)
    ).astype(NP_LP)


def _pack_vec(v):
    """[G] -> [128, G/128]: out[p, mc] = v[mc*128+p]."""
    v = np.asarray(v, np.float32)
    return np.ascontiguousarray(v.reshape(-1, 128).T)


def host_prep(inputs, t_steps=T_FULL):
    """Build the per-core in_map from the full (unsharded) inputs."""
    ip = {k: np.asarray(v, np.float32) if np.asarray(v).dtype != np.int64
          else np.asarray(v) for k, v in inputs.items()}
    m = {}
    m["emb"] = np.ascontiguousarray(ip["emb"].astype(np.float32))
    m["idx"] = np.stack(
        [
            ip["sentA"][:t_steps].astype(np.int32).reshape(-1, 1),
            ip["sentB"][:t_steps].astype(np.int32).reshape(-1, 1),
        ]
    )  # [2, t, 1]

    twoE = 2 * E
    rs_wih = []
    for l in range(NL):
        Whh = np.asarray(ip["Whh1"][l], np.float32)
        Wih = np.asarray(ip["Wih1"][l], np.float32)
        bih = np.asarray(ip["bih1"][l], np.float32)
        bhh = np.asarray(ip["bhh1"][l], np.float32)
        rs_hh = Whh.sum(axis=1)
        rs_ih = Wih.sum(axis=1)
        rs_wih.append(rs_ih)
        m[f"whh1_{l}"] = _pack_lhsT(_double_n(Whh))
        m[f"wih1_{l}"] = _pack_lhsT(_double_n(Wih))
        # gi bias consts: layer 0 epoch 0 consumes raw token embeddings; layer 1
        # (and both layers in epoch 1) consume p-form inputs (x = p - 1).
        b = np.empty(G, np.float32)
        b[:twoE] = bih[:twoE] + bhh[:twoE] - rs_hh[:twoE]
        b[twoE:] = 2.0 * bih[twoE:]
        if l == 1:
            b[:twoE] -= rs_ih[:twoE]
            b[twoE:] -= 2.0 * rs_ih[twoE:]
        m[f"b1f_{l}"] = _pack_vec(b)
        # n-path hh const (goes into psN via the indicator matmul)
        m[f"cn_{l}"] = np.ascontiguousarray(
            (2.0 * (bhh[twoE:] - rs_hh[twoE:])).reshape(KC, 128)
        )
    # epoch-1 layer-0 gi consts (p-form input)
    Whh0 = np.asarray(ip["Whh1"][0], np.float32)
    bih0 = np.asarray(ip["bih1"][0], np.float32)
    bhh0 = np.asarray(ip["bhh1"][0], np.float32)
    rs_hh0 = Whh0.sum(axis=1)
    e0 = np.empty(G, np.float32)
    e0[:twoE] = bih0[:twoE] + bhh0[:twoE] - rs_hh0[:twoE] - rs_wih[0][:twoE]
    e0[twoE:] = 2.0 * (bih0[twoE:] - rs_wih[0][twoE:])
    m["e0f"] = _pack_vec(e0)

    # indicator rhs for the psN const matmul: ind[c, c'*2+s] = (c == c')
    ind = np.zeros((KC, KC * 2), np.float32)
    for c in range(KC):
        ind[c, 2 * c] = 1.0
        ind[c, 2 * c + 1] = 1.0
    m["ind"] = ind

    # gru2: gi2 = m * rowsum(Wih2) + biases (host-precomputed rowsums)
    Wih2 = np.asarray(ip["Wih2"], np.float32)
    Whh2 = np.asarray(ip["Whh2"], np.float32)
    bih2 = np.asarray(ip["bih2"], np.float32)
    bhh2 = np.asarray(ip["bhh2"], np.float32)
    rs_ih2 = Wih2.sum(axis=1)
    rs_hh2 = Whh2.sum(axis=1)
    m["whh2"] = _pack_lhsT(_double_n(Whh2))
    s2 = np.empty(G, np.float32)
    s2[:twoE] = rs_ih2[:twoE]
    s2[twoE:] = 2.0 * rs_ih2[twoE:]
    m["s2f"] = _pack_vec(s2)
    c2 = np.empty(G, np.float32)
    c2[:twoE] = bih2[:twoE] + bhh2[:twoE] - rs_hh2[:twoE]
    c2[twoE:] = 2.0 * bih2[twoE:]
    m["c2f"] = _pack_vec(c2)
    m["cn_2"] = np.ascontiguousarray(
        (2.0 * (bhh2[twoE:] - rs_hh2[twoE:])).reshape(KC, 128)
    )

    # conv: wc[p, i*4+kc, o] = conv_w[o, i, kc*128+p]
    cw = np.asarray(ip["conv_w"], np.float32)  # [2, 2, 512]
    wc = cw.reshape(2, 2, 4, 128).transpose(3, 1, 2, 0).reshape(128, 8, 2)
    m["wc"] = np.ascontiguousarray(wc).astype(NP_LP)
    m["convb"] = np.asarray(ip["conv_b"], np.float32).reshape(2, 1)
    # double linear: hs = hx @ WA + hv @ WB + b_bi ; WA is [H, TEMP] = [K, M]
    m["wa"] = _pack_lhsT(ip["WA"].T)
    m["wb"] = _pack_lhsT(ip["WB"].T)
    m["bbi"] = _pack_vec(ip["b_bi"])  # [128, 2]
    # W_lin [1, 256]: wlin[p, kc, 0] = W_lin[0, kc*128+p]
    m["wlin"] = np.ascontiguousarray(
        np.asarray(ip["W_lin"], np.float32).reshape(2, 128).T.reshape(128, 2, 1)
    ).astype(NP_LP)
    m["blin"] = np.asarray(ip["b_lin"], np.float32).reshape(1, 1)
    return m


# ---------------------------------------------------------------------------
# device program


def _bcast(ap, extra):
    """append broadcast dims (step 0) to an AP"""
    return bass.AP(tensor=ap.tensor, offset=ap.offset, ap=list(ap.ap) + [[0, n] for n in extra])


def build_nc(t_steps=T_FULL, batch=16):
    _apply_tile_patch()
    assert t_steps % batch == 0
    lag = batch + 1
    nc = bass.Bass()

    def dparam(name, shape, dtype):
        return nc.declare_dram_parameter(name, list(shape), dtype, isOutput=False)

    emb = dparam("emb", [VOCAB, E], dt.float32)
    idx = dparam("idx", [2, t_steps, 1], dt.int32)
    w1d = [
        (dparam(f"wih1_{l}", [128, KC, MC, 128], W_DT), dparam(f"whh1_{l}", [128, KC, MC, 128], W_DT))
        for l in range(NL)
    ]
    b1d = [dparam(f"b1f_{l}", [128, MC], dt.float32) for l in range(NL)]
    cnd = [dparam(f"cn_{l}", [KC, 128], dt.float32) for l in range(NL)]
    e0f_d = dparam("e0f", [128, MC], dt.float32)
    ind_d = dparam("ind", [KC, KC * 2], dt.float32)
    whh2_d = dparam("whh2", [128, KC, MC, 128], W_DT)
    s2f_d = dparam("s2f", [128, MC], dt.float32)
    c2f_d = dparam("c2f", [128, MC], dt.float32)
    cn2_d = dparam("cn_2", [KC, 128], dt.float32)
    wc_d = dparam("wc", [128, 8, 2], W_DT)
    convb_d = dparam("convb", [2, 1], dt.float32)
    wa_d = dparam("wa", [128, KC, 2, 128], W_DT)
    wb_d = dparam("wb", [128, KC, 2, 128], W_DT)
    bbi_d = dparam("bbi", [128, 2], dt.float32)
    wlin_d = dparam("wlin", [128, 2, 1], W_DT)
    blin_d = dparam("blin", [1, 1], dt.float32)
    out_d = nc.declare_dram_parameter("out", [1, 1], dt.float32, isOutput=True)

    th_cnt = max(1, t_steps // 128)
    tok_pp = min(128, t_steps)  # tokens per indirect gather

    with tile.TileContext(nc) as tc, ExitStack() as ctx:
        P = ctx.enter_context(tc.tile_pool(name="persist", bufs=1))
        Wp = ctx.enter_context(tc.tile_pool(name="work", bufs=3))
        HP = ctx.enter_context(tc.tile_pool(name="hstate", bufs=3))
        DP = ctx.enter_context(tc.tile_pool(name="dram", bufs=1, space="DRAM"))

        # ---- input-dependent DMAs first so the gather isn't queued behind
        # ~8MB of weights ----
        idx_sb = P.tile([tok_pp, 2, th_cnt], dt.int32, tag="idx")
        for s in range(2):
            nc.gpsimd.dma_start(
                out=idx_sb[:, s, :],
                in_=idx[s, :, :].rearrange("(h p) o -> p (h o)", p=tok_pp),
            )
        gat = P.tile([tok_pp, th_cnt, 2, E], dt.float32, tag="gat")
        for s in range(2):
            for h in range(th_cnt):
                nc.gpsimd.indirect_dma_start(
                    out=gat[:, h, s, :],
                    out_offset=None,
                    in_=emb[:],
                    in_offset=bass.IndirectOffsetOnAxis(ap=idx_sb[:, s, h : h + 1], axis=0),
                )

        # ---- persistent SBUF: layer-0 weights first (needed first) ----
        w1_sb = []
        b1_sb = []
        cn_sb = []
        for l in range(NL):
            wi = P.tile([128, KC, MC, 128], W_DT, tag=f"wih1_{l}")
            wh = P.tile([128, KC, MC, 128], W_DT, tag=f"whh1_{l}")
            nc.gpsimd.dma_start(out=wi[:], in_=w1d[l][0][:])
            nc.gpsimd.dma_start(out=wh[:], in_=w1d[l][1][:])
            w1_sb.append((wi, wh))
            bf = P.tile([128, MC], dt.float32, tag=f"b1f_{l}")
            nc.gpsimd.dma_start(out=bf[:], in_=b1d[l][:])
            b1_sb.append(bf)
            cn = P.tile([KC, 128], dt.float32, tag=f"cn_{l}")
            nc.gpsimd.dma_start(out=cn[:], in_=cnd[l][:])
            cn_sb.append(cn)
        ind_sb = P.tile([KC, KC * 2], dt.float32, tag="ind")
        nc.gpsimd.dma_start(out=ind_sb[:], in_=ind_d[:])
        e0f_sb = P.tile([128, MC], dt.float32, tag="e0f")
        nc.gpsimd.dma_start(out=e0f_sb[:], in_=e0f_d[:])
        whh2_sb = P.tile([128, KC, MC, 128], W_DT, tag="whh2")
        nc.gpsimd.dma_start(out=whh2_sb[:], in_=whh2_d[:])
        s2f_sb = P.tile([128, MC], dt.float32, tag="s2f")
        nc.gpsimd.dma_start(out=s2f_sb[:], in_=s2f_d[:])
        c2f_sb = P.tile([128, MC], dt.float32, tag="c2f")
        nc.gpsimd.dma_start(out=c2f_sb[:], in_=c2f_d[:])
        cn2_sb = P.tile([KC, 128], dt.float32, tag="cn_2")
        nc.gpsimd.dma_start(out=cn2_sb[:], in_=cn2_d[:])
        wc_sb = P.tile([128, 8, 2], W_DT, tag="wc")
        nc.gpsimd.dma_start(out=wc_sb[:], in_=wc_d[:])
        convb_sb = P.tile([2, 1], dt.float32, tag="convb")
        nc.gpsimd.dma_start(out=convb_sb[:], in_=convb_d[:])
        wa_sb = P.tile([128, KC, 2, 128], W_DT, tag="wa")
        wb_sb = P.tile([128, KC, 2, 128], W_DT, tag="wb")
        nc.gpsimd.dma_start(out=wa_sb[:], in_=wa_d[:])
        nc.gpsimd.dma_start(out=wb_sb[:], in_=wb_d[:])
        bbi_sb = P.tile([128, 2], dt.float32, tag="bbi")
        nc.gpsimd.dma_start(out=bbi_sb[:], in_=bbi_d[:])
        wlin_sb = P.tile([128, 2, 1], W_DT, tag="wlin")
        nc.gpsimd.dma_start(out=wlin_sb[:], in_=wlin_d[:])
        blin_sb = P.tile([1, 1], dt.float32, tag="blin")
        nc.gpsimd.dma_start(out=blin_sb[:], in_=blin_d[:])

        ident = P.tile([128, 128], dt.float32, tag="ident")
        make_identity(nc, ident[:])
        ones_col = P.tile([128, 1], A_DT, tag="ones_col")
        nc.vector.memset(ones_col[:], 1.0)
        ones2 = P.tile([2, 128], W_DT, tag="ones2")
        nc.vector.memset(ones2[:], 1.0)
        pinit = P.tile([128, KC, 2], A_DT, tag="pinit")
        nc.vector.memset(pinit[:], 1.0)

        xT = P.tile([128, KC, 2, t_steps], A_DT, tag="xT")
        gi0 = P.tile([128, MC, 2, t_steps], dt.float32, tag="gi0")
        p0h = P.tile([128, KC, 2, t_steps], A_DT, tag="p0h")
        gi1 = P.tile([128, 2, MC, 2, batch], dt.float32, tag="gi1")

        # ================= phase A: transpose + gi0 =================
        with tc.tile_pool(name="psA", bufs=2, space="PSUM") as psA:
            for s in range(2):
                for h in range(th_cnt):
                    for c in range(KC):
                        tp = psA.tile([128, tok_pp], dt.float32, tag="tr")
                        nc.tensor.transpose(
                            out=tp[:],
                            in_=gat[:, h, s, c * 128 : (c + 1) * 128],
                            identity=ident[:tok_pp, :tok_pp],
                        )
                        nc.vector.tensor_copy(
                            out=xT[:, c, s, h * 128 : h * 128 + tok_pp], in_=tp[:]
                        )
            # gi0 = Wih1[0](n-doubled) @ x + b1f_0, gate-major
            for mc in range(MC):
                gp = psA.tile([128, 2, t_steps], dt.float32, tag="gi0p")
                for kc in range(KC):
                    nc.tensor.matmul(
                        out=gp[:],
                        lhsT=w1_sb[0][0][:, kc, mc, :],
                        rhs=xT[:, kc, :, :],
                        start=(kc == 0),
                        stop=(kc == KC - 1),
                    )
                nc.vector.tensor_scalar(
                    out=gi0[:, mc, :, :],
                    in0=gp[:],
                    scalar1=b1_sb[0][:, mc : mc + 1],
                    scalar2=None,
                    op0=ALU.add,
                )

        # ================= the fused GRU step =================
        def step_emit(PS, whh_sb, cn_t, gi_r_ap, gi_z_ap, gi_n2_ap, p_prev_fn,
                      p_prev_full, p_out_ap, tag, p_out_extra=None):
            """One GRU step for both sentences.

            PE order R -> N -> Z with three separate psum tiles so sigma_r can
            fire a third of the way into the matvec and the n-path chain
            (rhn2 -> npre2 -> sigma2 -> u -> p') starts as soon as psN lands.
            Scalar queue order is sigma_r, sigma_z, sigma2; vector ops are
            emitted in expected-ready order (the engines run in-order)."""
            psR = PS.tile([128, 4, 2], dt.float32, tag="r")
            psZ = PS.tile([128, 4, 2], dt.float32, tag="z")
            psN = PS.tile([128, 4, 2], dt.float32, tag="n")
            # Z first: all three sigmas fire in the first half of the matvec,
            # so the next step's psum-WAR (identity matmul vs this step's
            # sigma reader) never stalls the PE stream.
            nc.tensor.matmul(out=psZ[:], lhsT=ident[:], rhs=gi_z_ap,
                             start=True, stop=True, skip_group_check=True)
            for mc in range(4):
                for kc in range(KC):
                    nc.tensor.matmul(
                        out=psZ[:, mc, :], lhsT=whh_sb[:, kc, 4 + mc, :],
                        rhs=p_prev_fn(kc), start=False, stop=(kc == KC - 1),
                        skip_group_check=True,
                    )
            z_sb = Wp.tile([128, 4, 2], dt.float32, tag=f"sz{tag}")
            nc.scalar.activation(z_sb[:], psZ[:], ACT.Sigmoid)
            nc.tensor.matmul(out=psR[:], lhsT=ident[:], rhs=gi_r_ap,
                             start=True, stop=True, skip_group_check=True)
            for mc in range(4):
                for kc in range(KC):
                    nc.tensor.matmul(
                        out=psR[:, mc, :], lhsT=whh_sb[:, kc, mc, :],
                        rhs=p_prev_fn(kc), start=False, stop=(kc == KC - 1),
                        skip_group_check=True,
                    )
            r_sb = Wp.tile([128, 4, 2], dt.float32, tag=f"sr{tag}")
            nc.scalar.activation(r_sb[:], psR[:], ACT.Sigmoid)
            zp = Wp.tile([128, 4, 2], dt.float32, tag=f"zp{tag}")
            nc.vector.tensor_tensor(out=zp[:], in0=z_sb[:], in1=p_prev_full, op=ALU.mult)
            omz2 = Wp.tile([128, 4, 2], dt.float32, tag=f"om{tag}")
            nc.vector.tensor_scalar(
                out=omz2[:], in0=z_sb[:], scalar1=-2.0, scalar2=2.0,
                op0=ALU.mult, op1=ALU.add,
            )
            nc.tensor.matmul(out=psN[:], lhsT=cn_t[:], rhs=ind_sb[:],
                             start=True, stop=True, skip_group_check=True)
            for mc in range(4):
                for kc in range(KC):
                    nc.tensor.matmul(
                        out=psN[:, mc, :], lhsT=whh_sb[:, kc, 8 + mc, :],
                        rhs=p_prev_fn(kc), start=False, stop=(kc == KC - 1),
                        skip_group_check=True,
                    )
            rhn2 = Wp.tile([128, 4, 2], dt.float32, tag=f"rh{tag}")
            nc.vector.tensor_tensor(out=rhn2[:], in0=psN[:], in1=r_sb[:], op=ALU.mult)
            npre2 = Wp.tile([128, 4, 2], dt.float32, tag=f"np{tag}")
            nc.vector.tensor_tensor(out=npre2[:], in0=rhn2[:], in1=gi_n2_ap, op=ALU.add)
            sg = Wp.tile([128, 4, 2], dt.float32, tag=f"s{tag}")
            nc.scalar.activation(sg[:], npre2[:], ACT.Sigmoid)
            u = Wp.tile([128, 4, 2], dt.float32, tag=f"u{tag}")
            nc.vector.tensor_tensor(out=u[:], in0=sg[:], in1=omz2[:], op=ALU.mult)
            nc.vector.tensor_tensor(out=p_out_ap, in0=u[:], in1=zp[:], op=ALU.add)
            if p_out_extra is not None:
                nc.vector.tensor_tensor(out=p_out_extra, in0=u[:], in1=zp[:], op=ALU.add)

        # ================= phase B: the two interleaved scans =================
        p1_ring = [None]
        with tc.tile_pool(name="psB", bufs=2, space="PSUM") as psB:

            def l0_step(t):
                prev = pinit if t == 0 else None
                if prev is None:
                    pfn = lambda kc: p0h[:, kc, :, t - 1]
                    pfull = p0h[:, :, :, t - 1]
                else:
                    pfn = lambda kc: prev[:, kc, :]
                    pfull = prev[:, :, :]
                step_emit(
                    psB, w1_sb[0][1], cn_sb[0],
                    gi0[:, 0:4, :, t], gi0[:, 4:8, :, t], gi0[:, 8:12, :, t],
                    pfn, pfull, p0h[:, :, :, t], "a",
                )

            def gi1_batch(b):
                t0 = b * batch
                gp = psB.tile([128, MC, 2, batch], dt.float32, tag="g1")
                for mc in range(MC):
                    for kc in range(KC):
                        nc.tensor.matmul(
                            out=gp[:, mc, :, :],
                            lhsT=w1_sb[1][0][:, kc, mc, :],
                            rhs=p0h[:, kc, :, t0 : t0 + batch],
                            start=(kc == 0),
                            stop=(kc == KC - 1),
                        )
                nc.vector.tensor_tensor(
                    out=gi1[:, b % 2, :, :, :],
                    in0=gp[:],
                    in1=_bcast(b1_sb[1][:, :], [2, batch]),
                    op=ALU.add,
                )

            def l1_step(t):
                buf = (t // batch) % 2
                tb = t % batch
                prev = pinit if t == 0 else p1_ring[0]
                pn = HP.tile([128, KC, 2], A_DT, tag="p1")
                step_emit(
                    psB, w1_sb[1][1], cn_sb[1],
                    gi1[:, buf, 0:4, :, tb], gi1[:, buf, 4:8, :, tb],
                    gi1[:, buf, 8:12, :, tb],
                    lambda kc: prev[:, kc, :], prev[:, :, :], pn[:], "b",
                )
                p1_ring[0] = pn

            # Per-step sim-time floors (tile_wait_until) pin the scheduler to
            # the intended slot cadence: the Tile list-scheduler's cost model
            # underestimates the PE stream (ldweights is free there), so
            # without floors it queues step t+1's rz-sigmoid ahead of step
            # t's n-sigmoid on the in-order scalar engine, head-of-line
            # blocking every cell chain on real hardware.
            SLOT_MS = 0.003
            for t in range(t_steps):
                with tc.tile_wait_until(t * SLOT_MS):
                    l0_step(t)
                    if t % batch == batch - 1:
                        gi1_batch(t // batch)
                with tc.tile_wait_until((t + 0.5) * SLOT_MS):
                    if t >= lag:
                        l1_step(t - lag)
            for i, tp in enumerate(range(t_steps - lag, t_steps)):
                with tc.tile_wait_until((t_steps + i) * SLOT_MS):
                    l1_step(tp)

            # ============ epoch 1: seq len 2 over the finals (p-form) ============
            e1p = P.tile([128, KC, 2, 2], A_DT, tag="e1p")  # [.., step, sent]
            nc.vector.tensor_copy(out=e1p[:, :, 0, :], in_=p0h[:, :, :, t_steps - 1])
            nc.vector.tensor_copy(out=e1p[:, :, 1, :], in_=p1_ring[0][:, :, :])
            pf = []
            xcur = e1p
            for l in range(NL):
                gp = psB.tile([128, MC, 2, 2], dt.float32, tag="g1")
                for mc in range(MC):
                    for kc in range(KC):
                        nc.tensor.matmul(
                            out=gp[:, mc, :, :],
                            lhsT=w1_sb[l][0][:, kc, mc, :],
                            rhs=xcur[:, kc, :, :],
                            start=(kc == 0),
                            stop=(kc == KC - 1),
                        )
                gie = P.tile([128, MC, 2, 2], dt.float32, tag=f"gie{l}")
                bsrc = e0f_sb if l == 0 else b1_sb[1]
                nc.vector.tensor_tensor(
                    out=gie[:], in0=gp[:], in1=_bcast(bsrc[:, :], [2, 2]), op=ALU.add
                )
                xnext = P.tile([128, KC, 2, 2], A_DT, tag=f"e1y{l}")
                pe1f = HP.tile([128, KC, 2], A_DT, tag=f"pe{l}")
                step_emit(
                    psB, w1_sb[l][1], cn_sb[l],
                    gie[:, 0:4, 0, :], gie[:, 4:8, 0, :], gie[:, 8:12, 0, :],
                    lambda kc: pinit[:, kc, :], pinit[:, :, :],
                    xnext[:, :, 0, :], f"e{l}",
                )
                step_emit(
                    psB, w1_sb[l][1], cn_sb[l],
                    gie[:, 0:4, 1, :], gie[:, 4:8, 1, :], gie[:, 8:12, 1, :],
                    lambda kc: xnext[:, kc, 0, :], xnext[:, :, 0, :],
                    xnext[:, :, 1, :], f"e{l}", p_out_extra=pe1f[:],
                )
                pf.append(pe1f)
                xcur = xnext

            # hE[i] = pE[i] - 1  (fp32)
            hE = []
            for i in range(2):
                he = Wp.tile([128, KC, 2], dt.float32, tag=f"hE{i}")
                nc.vector.tensor_scalar(
                    out=he[:], in0=pf[i][:], scalar1=-1.0, scalar2=None, op0=ALU.add
                )
                hE.append(he)

        # ================= phase C: conv + maxpool + gru2 + head =================
        with tc.tile_pool(name="psC1", bufs=1, space="PSUM") as psC:
            PADL = 255
            hp_dram = DP.tile([4, E + 2 * PADL], A_DT)  # rows (i, s)
            zs = P.tile([4, E + 2 * PADL], A_DT, tag="zs")
            nc.vector.memset(zs[:], 0.0)
            nc.gpsimd.dma_start(out=hp_dram[:], in_=zs[:])
            # flatten hE via PE transpose, write rows straight into the pad buffer
            for i in range(2):
                for s in range(2):
                    tp = psC.tile([KC, 128], dt.float32, tag="tr2")
                    nc.tensor.transpose(out=tp[:], in_=hE[i][:, :, s], identity=ident[:])
                    trs = Wp.tile([KC, 128], A_DT, tag="trs")
                    nc.vector.tensor_copy(out=trs[:], in_=tp[:])
                    r = 2 * i + s
                    nc.gpsimd.dma_start(
                        out=hp_dram[r : r + 1, PADL : PADL + E].rearrange(
                            "o (c f) -> (o c) f", c=KC
                        ),
                        in_=trs[:],
                    )
            # im2col: load time-major rows Hrow[t, k] = hp[2t + k] (contiguous
            # along k), then PE-transpose to K-major Hcol[(i,k), (s,t)].
            Hcol = P.tile([128, 8, 2, 256], A_DT, tag="Hcol")
            ident_lp = P.tile([128, 128], A_DT, tag="ident_lp")
            make_identity(nc, ident_lp[:])
            hp_base = hp_dram[:]
            row_sz = E + 2 * PADL
            for i in range(2):
                for s in range(2):
                    for th in range(2):
                        hrow = Wp.tile([128, E], A_DT, tag="Hrow")
                        src = bass.AP(
                            tensor=hp_base.tensor,
                            offset=hp_base.offset + (i * 2 + s) * row_sz + 256 * th,
                            ap=[[2, 128], [1, E]],
                        )
                        nc.gpsimd.dma_start(out=hrow[:], in_=src)
                        for kc in range(KC):
                            tpb = psC.tile([128, 128], A_DT, tag="tr3")
                            nc.tensor.transpose(
                                out=tpb[:],
                                in_=hrow[:, kc * 128 : (kc + 1) * 128],
                                identity=ident_lp[:],
                            )
                            nc.vector.tensor_copy(
                                out=Hcol[:, i * KC + kc, s, th * 128 : (th + 1) * 128],
                                in_=tpb[:],
                            )
            # conv matmul -> [2(out_ch), 2(s), 256(t)] then global max per (o, s)
            cp = psC.tile([2, 2, 256], dt.float32, tag="conv")
            for ck in range(8):
                nc.tensor.matmul(
                    out=cp[:],
                    lhsT=wc_sb[:, ck, :],
                    rhs=Hcol[:, ck, :, :],
                    start=(ck == 0),
                    stop=(ck == 7),
                )
            mx = Wp.tile([2, 2, 1], dt.float32, tag="mx")
            nc.vector.tensor_reduce(out=mx[:], in_=cp[:], axis=mybir.AxisListType.X, op=ALU.max)
            m_sb = Wp.tile([2, 2], dt.float32, tag="m_sb")
            nc.vector.tensor_scalar(
                out=m_sb[:],
                in0=mx[:, :, 0],
                scalar1=convb_sb[:, 0:1],
                scalar2=None,
                op0=ALU.add,
            )
            # broadcast m over partitions via ones-matmul with a diagonal rhs
            m_lp = Wp.tile([2, 2], A_DT, tag="m_lp")
            nc.vector.tensor_copy(out=m_lp[:], in_=m_sb[:])
            md = Wp.tile([2, 4], A_DT, tag="md")
            nc.vector.memset(md[:], 0.0)
            # diagonal placement via DMA (compute engines can't address odd
            # base partitions)
            nc.gpsimd.dma_start(out=md[0:1, 0:2], in_=m_lp[0:1, :])
            nc.gpsimd.dma_start(out=md[1:2, 2:4], in_=m_lp[1:2, :])
            mp = psC.tile([128, 4], dt.float32, tag="mbc")
            nc.tensor.matmul(out=mp[:], lhsT=ones2[:], rhs=md[:], start=True, stop=True)
            mB = Wp.tile([128, 4], dt.float32, tag="mB")
            nc.vector.tensor_copy(out=mB[:], in_=mp[:])

        with tc.tile_pool(name="psC2", bufs=1, space="PSUM") as psC:
            # gi2[tp] = m[tp] * s2f + c2f  (both gates pre-scaled; n-part doubled)
            gi2 = P.tile([128, MC, 2, 2], dt.float32, tag="gi2")  # [.., step, sent]
            for tpp in range(2):
                for s in range(2):
                    nc.vector.scalar_tensor_tensor(
                        out=gi2[:, :, tpp, s],
                        in0=s2f_sb[:],
                        scalar=mB[:, 2 * tpp + s : 2 * tpp + s + 1],
                        in1=c2f_sb[:],
                        op0=ALU.mult,
                        op1=ALU.add,
                    )
            # gru2: 2 steps
            p2a = HP.tile([128, KC, 2], A_DT, tag="p2a")
            step_emit(
                psC, whh2_sb, cn2_sb,
                gi2[:, 0:4, 0, :], gi2[:, 4:8, 0, :], gi2[:, 8:12, 0, :],
                lambda kc: pinit[:, kc, :], pinit[:, :, :], p2a[:], "g2",
            )
            p2b = HP.tile([128, KC, 2], A_DT, tag="p2b")
            step_emit(
                psC, whh2_sb, cn2_sb,
                gi2[:, 0:4, 1, :], gi2[:, 4:8, 1, :], gi2[:, 8:12, 1, :],
                lambda kc: p2a[:, kc, :], p2a[:, :, :], p2b[:], "g2",
            )
            h2 = Wp.tile([128, KC, 2], dt.float32, tag="h2")
            nc.vector.tensor_scalar(
                out=h2[:], in0=p2b[:], scalar1=-1.0, scalar2=None, op0=ALU.add
            )
            # head: hx = hA*hB, hv = |hA-hB|
            hx = Wp.tile([128, KC], dt.float32, tag="hx")
            nc.vector.tensor_tensor(out=hx[:], in0=h2[:, :, 0], in1=h2[:, :, 1], op=ALU.mult)
            hv0 = Wp.tile([128, KC], dt.float32, tag="hv0")
            nc.vector.tensor_tensor(out=hv0[:], in0=h2[:, :, 0], in1=h2[:, :, 1], op=ALU.subtract)
            hv = Wp.tile([128, KC], dt.float32, tag="hv")
            nc.scalar.activation(hv[:], hv0[:], ACT.Abs)
            hx_lp = Wp.tile([128, KC], A_DT, tag="hx_lp")
            hv_lp = Wp.tile([128, KC], A_DT, tag="hv_lp")
            nc.vector.tensor_copy(out=hx_lp[:], in_=hx[:])
            nc.vector.tensor_copy(out=hv_lp[:], in_=hv[:])
            hsp = psC.tile([128, 2], dt.float32, tag="hs")
            for mc in range(2):
                for kc in range(KC):
                    nc.tensor.matmul(
                        out=hsp[:, mc : mc + 1],
                        lhsT=wa_sb[:, kc, mc, :],
                        rhs=hx_lp[:, kc : kc + 1],
                        start=(kc == 0),
                        stop=False,
                    )
                for kc in range(KC):
                    nc.tensor.matmul(
                        out=hsp[:, mc : mc + 1],
                        lhsT=wb_sb[:, kc, mc, :],
                        rhs=hv_lp[:, kc : kc + 1],
                        start=False,
                        stop=(kc == KC - 1),
                    )
            hspre = Wp.tile([128, 2], dt.float32, tag="hspre")
            nc.vector.tensor_tensor(out=hspre[:], in0=hsp[:], in1=bbi_sb[:], op=ALU.add)
            ht = Wp.tile([128, 2], dt.float32, tag="ht")
            nc.scalar.activation(ht[:], hspre[:], ACT.Tanh)
            ht_lp = Wp.tile([128, 2], A_DT, tag="ht_lp")
            nc.vector.tensor_copy(out=ht_lp[:], in_=ht[:])
            op = psC.tile([1, 1], dt.float32, tag="out")
            for kc in range(2):
                nc.tensor.matmul(
                    out=op[:],
                    lhsT=wlin_sb[:, kc, :],
                    rhs=ht_lp[:, kc : kc + 1],
                    start=(kc == 0),
                    stop=(kc == 1),
                )
            out_sb = Wp.tile([1, 1], dt.float32, tag="osb")
            nc.scalar.activation(out_sb[:], op[:], ACT.Sigmoid, bias=blin_sb[:])
            nc.gpsimd.dma_start(out=out_d[:], in_=out_sb[:])

    _legalize_waits(nc)
    return nc


# ---------------------------------------------------------------------------
_NC_CACHE = {}


def _get_nc(t_steps=T_FULL, batch=16):
    key = (t_steps, batch)
    if key not in _NC_CACHE:
        _NC_CACHE[key] = build_nc(t_steps, batch)
    return _NC_CACHE[key]


def run(inputs, t_steps=T_FULL, batch=16, trace=False):
    nc = _get_nc(t_steps, batch)
    in_map = host_prep(inputs, t_steps)
    res = run_bass_kernel_spmd(nc, [in_map] * N_CORES, list(range(N_CORES)), trace=trace)
    out = np.asarray(res.results[0]["out"], np.float32)
    return out, res


def kernel(**inputs) -> np.ndarray:
    out, _ = run(inputs)
    return out
